# revision 9
# baseline (speedup 1.0000x reference)
"""Causal self-attention (B=4, T=2048, C=1024, H=16) on 8 TRN2 NeuronCores.

Sharding: 8 cores = 4 batches x 2 head-groups (8 heads each). Core c = g*4+b
handles batch b, heads 8g..8g+8 (4 pairs of 2). Inside kernel(): the host
transposes x[b] -> xT [C,T] (bf16), slices/arranges W_attn columns (Wq
pre-scaled by 1/sqrt(D)) and W_proj rows per group (bf16), runs one Bass/Tile
kernel SPMD on cores 0-7, then sums the two group-partial out^T [C,T] per
batch and transposes.

Per-core device pipeline (all matmuls bf16, PSUM f32):
  1. QK^T projection -> Q^T/K^T [128(2 heads), T] per pair (per-k weight DMA
     chunks so the first matmul starts ~4us in)
  2. V in natural layout [tk, head, 64] + fused ones column (softmax denom)
  3. per (pair, head): flat chunk pipeline over (jt, chunk): scores^T =
     K_jt @ Q^T over the causal span (f32 PSUM) -> ACT exp -> es (bf16 SBUF)
     -> diagonal mask-mul; PV of the PREVIOUS chunk is emitted after the
     current chunk's scores so the PE never head-of-line blocks on exp.
  4. drain Y^T+sums [65,T] to SBUF fast (frees the PSUM accumulator), then
     recip(sums) -> gpsimd partition_broadcast -> DVE mul -> ysb (bf16)
  5. output projection: out^T[cout,n] = sum_kp Wp_kp^T @ Y^T_kp -> DMA out
"""
import sys
if '/opt/trn_rl_repo' not in sys.path:
    sys.path.insert(0, '/opt/trn_rl_repo')
import numpy as np
import ml_dtypes
import concourse.bacc as bacc
import concourse.tile as tile
import concourse.mybir as mybir
from concourse import bass_utils

F32 = mybir.dt.float32
BF16 = mybir.dt.bfloat16
BF16_NP = ml_dtypes.bfloat16

N_EMBED = 1024
N_HEAD = 16
D = 64
B_FULL, T_FULL, C_FULL = 4, 2048, 1024
N_GROUPS = 2


def build_kernel(T=T_FULL, C=C_FULL, n_pairs=4, reps=1, n_strip=512):
    HP = n_pairs * 2
    CIN = HP * D
    n_k = C // 128
    n_jt = T // 128
    n_ts = T // n_strip
    jt_per_strip = n_strip // 128
    CH = min(2 * n_strip, T)

    nc = bacc.Bacc("TRN2", target_bir_lowering=False, debug=False)
    xt_d = nc.dram_tensor("xt", [C, T], BF16, kind="ExternalInput")
    wqk_d = nc.dram_tensor("wqk", [C, n_pairs * 2 * 128], BF16, kind="ExternalInput")
    wv_d = nc.dram_tensor("wv", [C, n_pairs * 128], BF16, kind="ExternalInput")
    wp_d = nc.dram_tensor("wp", [CIN, C], BF16, kind="ExternalInput")
    mask_d = nc.dram_tensor("mask", [128, 128], BF16, kind="ExternalInput")
    outp_d = nc.dram_tensor("outp", [C, T], BF16, kind="ExternalOutput")

    xt_r = xt_d.ap().rearrange("(k p) t -> p k t", p=128)
    wqk_r = wqk_d.ap().rearrange("(k p) m -> p k m", p=128)
    wv_r = wv_d.ap().rearrange("(k p) m -> p k m", p=128)
    wp_r = wp_d.ap().rearrange("(k p) m -> p k m", p=128)

    with tile.TileContext(nc) as tc:
        with tc.tile_pool(name="bigp", bufs=1) as bigp, \
             tc.tile_pool(name="wvp", bufs=1) as wvp, \
             tc.tile_pool(name="xwp", bufs=2) as xwp, \
             tc.tile_pool(name="qkp", bufs=1) as qkp, \
             tc.tile_pool(name="vp", bufs=1) as vp, \
             tc.tile_pool(name="maskp", bufs=1) as maskp, \
             tc.tile_pool(name="esp", bufs=3) as esp, \
             tc.tile_pool(name="normp", bufs=2) as normp, \
             tc.tile_pool(name="osbp", bufs=2) as osbp, \
             tc.tile_pool(name="ps_a", bufs=2, space="PSUM") as ps_a, \
             tc.tile_pool(name="ps_y", bufs=1, space="PSUM") as ps_y:

            def body(_i=None, unroll=1):
                mask_sb = maskp.tile([128, 128], BF16)
                nc.sync.dma_start(out=mask_sb[:], in_=mask_d.ap())
                scr = maskp.tile([1, 1], F32, tag="scr")
                # preload the exp table set during phase 1
                nc.scalar.activation(
                    out=scr[:], in_=mask_sb[0:1, 0:1],
                    func=mybir.ActivationFunctionType.Exp)

                # strip-0 x first so the first projection matmul starts early
                xs_tiles = []
                xs0 = xwp.tile([128, n_k, n_strip], BF16, tag="xw")
                nc.sync.dma_start(out=xs0[:], in_=xt_r[:, :, 0:n_strip])
                xs_tiles.append(xs0)

                wqk_sb = bigp.tile([128, n_k, n_pairs * 2 * 128], BF16, tag="big")
                for k in range(n_k):
                    nc.sync.dma_start(out=wqk_sb[:, k, :], in_=wqk_r[:, k, :])
                wv_sb = wvp.tile([128, n_k, n_pairs * 128], BF16)
                for k in range(n_k):
                    nc.sync.dma_start(out=wv_sb[:, k, :], in_=wv_r[:, k, :])

                qt = [qkp.tile([128, T], BF16, tag=f"qt{p}", name=f"qt{p}")
                      for p in range(n_pairs)]
                kt = [qkp.tile([128, T], BF16, tag=f"kt{p}", name=f"kt{p}")
                      for p in range(n_pairs)]
                v_aug = vp.tile([128, n_jt, HP, 65], BF16)
                nc.vector.memset(v_aug[:], 1.0)

                # ---- phase 1: projections, streamed over token strips ----
                for s in range(n_ts):
                    sl = slice(s * n_strip, (s + 1) * n_strip)
                    if s == 0:
                        xs = xs_tiles[0]
                    else:
                        xs = xwp.tile([128, n_k, n_strip], BF16, tag="xw")
                        nc.sync.dma_start(out=xs[:], in_=xt_r[:, :, sl])
                    for p in range(n_pairs):
                        for qk in range(2):
                            ps = ps_a.tile([128, n_strip], F32, tag="a")
                            for k in range(n_k):
                                nc.tensor.matmul(
                                    ps[:],
                                    wqk_sb[:, k, (p * 2 + qk) * 128:(p * 2 + qk + 1) * 128],
                                    xs[:, k, :],
                                    start=(k == 0), stop=(k == n_k - 1))
                            dst = (qt if qk == 0 else kt)[p]
                            nc.vector.tensor_copy(dst[:, sl], ps[:])
                    for nt in range(jt_per_strip):
                        psv = ps_a.tile([128, n_pairs * 128], F32, tag="a")
                        for k in range(n_k):
                            nc.tensor.matmul(
                                psv[:], xs[:, k, nt * 128:(nt + 1) * 128],
                                wv_sb[:, k, :],
                                start=(k == 0), stop=(k == n_k - 1))
                        jt = s * jt_per_strip + nt
                        nc.vector.tensor_copy(
                            v_aug[:, jt, :, 0:64],
                            psv[:].rearrange("q (h d) -> q h d", d=D))

                # wp for phase 3: DMA early so it overlaps phase 2
                wp_sb = xwp.tile([128, CIN // 128, C], BF16, tag="xw")
                nc.sync.dma_start(out=wp_sb[:], in_=wp_r)

                # ---- phase 2: attention per (pair, head) ----
                ysb = bigp.tile([128, n_pairs, T], BF16, tag="big")
                for p in range(n_pairs):
                    for h in range(2):
                        hh = p * 2 + h
                        hs = slice(h * 64, (h + 1) * 64)
                        y_ps = ps_y.tile([65, T], F32, tag="y")
                        # flat chunk list: (jt, c)
                        chunks = [(jt, c)
                                  for jt in range(n_jt)
                                  for c in range((128 * jt) // CH, T // CH)]
                        es_tiles = {}
                        pend = None

                        def emit_pv(jt, c):
                            es = es_tiles[jt]
                            lo = 128 * jt
                            s0 = jt // jt_per_strip
                            cw_lo = max(lo, c * CH)
                            full_start = (s0 + 1 if cw_lo == lo
                                          else (c * CH) // n_strip)
                            for s in range(full_start, (c + 1) * CH // n_strip):
                                nc.tensor.matmul(
                                    y_ps[:, s * n_strip:(s + 1) * n_strip],
                                    v_aug[:, jt, hh, :],
                                    es[:, s * n_strip:(s + 1) * n_strip],
                                    start=(jt == 0),
                                    stop=(jt == (s + 1) * jt_per_strip - 1),
                                    skip_group_check=True)
                            if cw_lo == lo:
                                off = 128 * jt - n_strip * s0
                                pv_n = min(n_strip - off, T - lo)
                                nc.tensor.matmul(
                                    y_ps[:, lo:lo + pv_n],
                                    v_aug[:, jt, hh, :], es[:, lo:lo + pv_n],
                                    start=(jt == 0),
                                    stop=(jt == (s0 + 1) * jt_per_strip - 1),
                                    skip_group_check=True)

                        for (jt, c) in chunks:
                            lo = 128 * jt
                            cw_lo = max(lo, c * CH)
                            if jt not in es_tiles:
                                es_tiles[jt] = esp.tile(
                                    [128, T], BF16, tag="es", name=f"es{jt % 3}")
                                if len(es_tiles) > 2:
                                    del es_tiles[min(k2 for k2 in es_tiles
                                                     if k2 != jt)]
                            es = es_tiles[jt]
                            scores = ps_a.tile([128, CH], F32, tag="a")
                            s_first = cw_lo // n_strip
                            for s in range(s_first, (c + 1) * CH // n_strip):
                                a = max(cw_lo, s * n_strip)
                                n = (s + 1) * n_strip - a
                                nc.tensor.matmul(
                                    scores[:, a - c * CH:a - c * CH + n],
                                    kt[p][hs, lo:lo + 128],
                                    qt[p][hs, a:a + n],
                                    start=True, stop=True)
                            nc.scalar.activation(
                                out=es[:, cw_lo:(c + 1) * CH],
                                in_=scores[:, cw_lo - c * CH:CH],
                                func=mybir.ActivationFunctionType.Exp)
                            if cw_lo == lo:
                                nc.vector.tensor_mul(
                                    es[:, lo:lo + 128], es[:, lo:lo + 128],
                                    mask_sb[:])
                            if pend is not None:
                                emit_pv(*pend)
                            pend = (jt, c)
                        emit_pv(*pend)

                        # fast drain to SBUF, then normalize off-PSUM
                        ydr = normp.tile([65, T], F32, tag="ydr")
                        nc.vector.tensor_copy(ydr[:], y_ps[:])
                        recip = normp.tile([1, T], F32, tag="recip")
                        nc.vector.reciprocal(recip[:], ydr[64:65, :])
                        bcast = normp.tile([64, T], F32, tag="bcast")
                        nc.gpsimd.partition_broadcast(bcast[:], recip[:])
                        nc.vector.tensor_mul(
                            ysb[h * 64:(h + 1) * 64, p, :], ydr[0:64, :],
                            bcast[:])

                # ---- phase 3: output projection (2 strips per PSUM tile) ----
                for m in range(C // 128):
                    for s2 in range(n_ts // 2):
                        sl = slice(s2 * 2 * n_strip, (s2 + 1) * 2 * n_strip)
                        pso = ps_a.tile([128, 2 * n_strip], F32, tag="a")
                        for half in range(2):
                            hsl = slice(half * n_strip, (half + 1) * n_strip)
                            ssl = slice((s2 * 2 + half) * n_strip,
                                        (s2 * 2 + half + 1) * n_strip)
                            for kp in range(CIN // 128):
                                nc.tensor.matmul(
                                    pso[:, hsl],
                                    wp_sb[:, kp, m * 128:(m + 1) * 128],
                                    ysb[:, kp, ssl],
                                    start=(kp == 0), stop=(kp == CIN // 128 - 1))
                        osb = osbp.tile([128, 2 * n_strip], BF16, tag="osb")
                        if s2 % 2 == 0:
                            nc.vector.tensor_copy(osb[:], pso[:])
                        else:
                            nc.scalar.copy(osb[:], pso[:])
                        nc.sync.dma_start(
                            out=outp_d.ap()[m * 128:(m + 1) * 128, sl], in_=osb[:])

            if reps == 1:
                body()
            else:
                with tc.For_i(0, reps, 1) as i:
                    body(i)
    nc.compile()
    return nc


def host_inputs(x, W_attn, W_proj, n_groups=N_GROUPS):
    """Per-core input maps. Core order: g * B + b."""
    B, T, C = x.shape
    hp = N_HEAD // n_groups
    n_pairs = hp // 2
    scale = np.float32(1.0 / np.sqrt(D))
    mask = (np.arange(128)[None, :] >= np.arange(128)[:, None]).astype(BF16_NP)
    in_maps = []
    for g in range(n_groups):
        qk_cols, v_cols = [], []
        for p in range(n_pairs):
            h0 = g * hp + 2 * p
            h1 = h0 + 1
            qk_cols.append(W_attn[:, h0 * D:(h0 + 1) * D] * scale)
            qk_cols.append(W_attn[:, h1 * D:(h1 + 1) * D] * scale)
            qk_cols.append(W_attn[:, C + h0 * D:C + (h0 + 1) * D])
            qk_cols.append(W_attn[:, C + h1 * D:C + (h1 + 1) * D])
            v_cols.append(W_attn[:, 2 * C + h0 * D:2 * C + (h0 + 1) * D])
            v_cols.append(W_attn[:, 2 * C + h1 * D:2 * C + (h1 + 1) * D])
        wqk = np.ascontiguousarray(
            np.concatenate(qk_cols, axis=1)).astype(BF16_NP)
        wv = np.ascontiguousarray(
            np.concatenate(v_cols, axis=1)).astype(BF16_NP)
        wp = np.ascontiguousarray(
            W_proj[g * hp * D:(g + 1) * hp * D]).astype(BF16_NP)
        for b in range(B):
            xt = np.ascontiguousarray(x[b].T).astype(BF16_NP)
            in_maps.append({"xt": xt, "wqk": wqk, "wv": wv, "wp": wp, "mask": mask})
    return in_maps


def host_gather(results, B, T, C, n_groups=N_GROUPS):
    out = np.zeros((B, T, C), dtype=np.float32)
    for g in range(n_groups):
        for b in range(B):
            out[b] += results[g * B + b]["outp"].T.astype(np.float32)
    return out


_NC_CACHE = {}


def kernel(x, W_attn, W_proj):
    x = np.asarray(x, dtype=np.float32)
    W_attn = np.asarray(W_attn, dtype=np.float32)
    W_proj = np.asarray(W_proj, dtype=np.float32)
    B, T, C = x.shape
    if "nc" not in _NC_CACHE:
        _NC_CACHE["nc"] = build_kernel(T=T, C=C)
    nc = _NC_CACHE["nc"]
    in_maps = host_inputs(x, W_attn, W_proj)
    res = bass_utils.run_bass_kernel_spmd(nc, in_maps, core_ids=list(range(8)))
    return host_gather(res.results, B, T, C)


# revision 13
# speedup vs baseline: 1.0474x; 1.0474x over previous
"""Causal self-attention (B=4, T=2048, C=1024, H=16) on 8 TRN2 NeuronCores.

Sharding: 8 cores = 4 batches x 2 head-groups (8 heads each). Core c = g*4+b
handles batch b, heads 8g..8g+8 (4 pairs of 2). Inside kernel(): the host
transposes x[b] -> xT [C,T] (bf16), slices/arranges W_attn columns (Wq
pre-scaled by 1/sqrt(D)) and W_proj rows per group (bf16), runs one Bass/Tile
kernel SPMD on cores 0-7, then sums the two group-partial out^T [C,T] per
batch and transposes.

Per-core device pipeline (all matmuls bf16, PSUM f32):
  1. Q^T/K^T projection for all 4 token strips (per-k weight DMA chunks so
     the first matmul starts ~4us in); V projection is deferred.
  2. attention per (pair, head): flat chunk pipeline over (jt, chunk):
     scores^T = K_jt @ Q^T over the causal span (f32 PSUM) -> ACT exp ->
     es (bf16 SBUF) -> diagonal mask-mul; the PREVIOUS chunk's PV is emitted
     after the current chunk's scores so the PE never blocks on exp. V-proj
     for key-tile jt is emitted just-in-time before its first PV, so the ACT
     exp stream starts right after Q/K instead of after all projections.
  3. drain Y^T+sums [65,T] to SBUF fast (frees the PSUM accumulator), then
     recip(sums) -> gpsimd partition_broadcast -> DVE mul -> ysb (bf16)
  4. output projection: out^T[m,:] -> osb bf16 -> one 512KB DMA per m-tile
     on the ACT HWDGE ring (overlaps next-rep input DMAs on the SP ring)
"""
import sys
if '/opt/trn_rl_repo' not in sys.path:
    sys.path.insert(0, '/opt/trn_rl_repo')
import numpy as np
import ml_dtypes
import concourse.bacc as bacc
import concourse.tile as tile
import concourse.mybir as mybir
from concourse import bass_utils

F32 = mybir.dt.float32
BF16 = mybir.dt.bfloat16
BF16_NP = ml_dtypes.bfloat16

N_EMBED = 1024
N_HEAD = 16
D = 64
B_FULL, T_FULL, C_FULL = 4, 2048, 1024
N_GROUPS = 2


def build_kernel(T=T_FULL, C=C_FULL, n_pairs=4, reps=1, n_strip=512, phases=3):
    HP = n_pairs * 2
    CIN = HP * D
    n_k = C // 128
    n_jt = T // 128
    n_ts = T // n_strip
    jt_per_strip = n_strip // 128
    CH = min(2 * n_strip, T)

    nc = bacc.Bacc("TRN2", target_bir_lowering=False, debug=False)
    xt_d = nc.dram_tensor("xt", [C, T], BF16, kind="ExternalInput")
    wqk_d = nc.dram_tensor("wqk", [C, n_pairs * 2 * 128], BF16, kind="ExternalInput")
    wv_d = nc.dram_tensor("wv", [C, n_pairs * 128], BF16, kind="ExternalInput")
    wp_d = nc.dram_tensor("wp", [CIN, C], BF16, kind="ExternalInput")
    mask_d = nc.dram_tensor("mask", [128, 128], BF16, kind="ExternalInput")
    outp_d = nc.dram_tensor("outp", [C, T], BF16, kind="ExternalOutput")

    xt_r = xt_d.ap().rearrange("(k p) t -> p k t", p=128)
    wqk_r = wqk_d.ap().rearrange("(k p) m -> p k m", p=128)
    wv_r = wv_d.ap().rearrange("(k p) m -> p k m", p=128)
    wp_r = wp_d.ap().rearrange("(k p) m -> p k m", p=128)

    with tile.TileContext(nc) as tc:
        with tc.tile_pool(name="bigp", bufs=1) as bigp, \
             tc.tile_pool(name="wvp", bufs=1) as wvp, \
             tc.tile_pool(name="xwp", bufs=4) as xwp, \
             tc.tile_pool(name="wpp", bufs=1) as wpp, \
             tc.tile_pool(name="ysbp", bufs=1) as ysbp, \
             tc.tile_pool(name="qkp", bufs=1) as qkp, \
             tc.tile_pool(name="vp", bufs=1) as vp, \
             tc.tile_pool(name="maskp", bufs=1) as maskp, \
             tc.tile_pool(name="esp", bufs=3) as esp, \
             tc.tile_pool(name="normp", bufs=2) as normp, \
             tc.tile_pool(name="osbp", bufs=2) as osbp, \
             tc.tile_pool(name="ps_a", bufs=2, space="PSUM") as ps_a, \
             tc.tile_pool(name="ps_y", bufs=1, space="PSUM") as ps_y:

            def body(_i=None, unroll=1):
                mask_sb = maskp.tile([128, 128], BF16)
                nc.sync.dma_start(out=mask_sb[:], in_=mask_d.ap())
                scr = maskp.tile([1, 1], F32, tag="scr")
                # preload the exp table set during phase 1
                nc.scalar.activation(
                    out=scr[:], in_=mask_sb[0:1, 0:1],
                    func=mybir.ActivationFunctionType.Exp)

                # strip-0 x first so the first projection matmul starts early
                xs_tiles = []
                xs0 = xwp.tile([128, n_k, n_strip], BF16, tag="xw", name="xs0")
                nc.sync.dma_start(out=xs0[:], in_=xt_r[:, :, 0:n_strip])
                xs_tiles.append(xs0)

                wqk_sb = bigp.tile([128, n_k, n_pairs * 2 * 128], BF16, tag="big")
                for k in range(n_k):
                    nc.sync.dma_start(out=wqk_sb[:, k, :], in_=wqk_r[:, k, :])
                wv_sb = wvp.tile([128, n_k, n_pairs * 128], BF16)
                for k in range(n_k):
                    nc.sync.dma_start(out=wv_sb[:, k, :], in_=wv_r[:, k, :])
                for s in range(1, n_ts):
                    xs = xwp.tile([128, n_k, n_strip], BF16, tag="xw",
                                  name=f"xs{s}")
                    nc.sync.dma_start(
                        out=xs[:], in_=xt_r[:, :, s * n_strip:(s + 1) * n_strip])
                    xs_tiles.append(xs)
                wp_sb = wpp.tile([128, CIN // 128, C], BF16)
                nc.sync.dma_start(out=wp_sb[:], in_=wp_r)

                qt = [qkp.tile([128, T], BF16, tag=f"qt{p}", name=f"qt{p}")
                      for p in range(n_pairs)]
                kt = [qkp.tile([128, T], BF16, tag=f"kt{p}", name=f"kt{p}")
                      for p in range(n_pairs)]
                v_aug = vp.tile([128, n_jt, HP, 65], BF16)
                nc.vector.memset(v_aug[:, :, :, 64:65], 1.0)

                # ---- phase 1: Q/K projections over token strips ----
                for s in range(n_ts):
                    sl = slice(s * n_strip, (s + 1) * n_strip)
                    xs = xs_tiles[s]
                    for p in range(n_pairs):
                        for qk in range(2):
                            ps = ps_a.tile([128, n_strip], F32, tag="a")
                            for k in range(n_k):
                                nc.tensor.matmul(
                                    ps[:],
                                    wqk_sb[:, k, (p * 2 + qk) * 128:(p * 2 + qk + 1) * 128],
                                    xs[:, k, :],
                                    start=(k == 0), stop=(k == n_k - 1))
                            dst = (qt if qk == 0 else kt)[p]
                            nc.vector.tensor_copy(dst[:, sl], ps[:])

                # V projection for one key-tile, emitted just-in-time
                def emit_v(jt):
                    xs = xs_tiles[jt // jt_per_strip]
                    nt = jt % jt_per_strip
                    psv = ps_a.tile([128, n_pairs * 128], F32, tag="a")
                    for k in range(n_k):
                        nc.tensor.matmul(
                            psv[:], xs[:, k, nt * 128:(nt + 1) * 128],
                            wv_sb[:, k, :],
                            start=(k == 0), stop=(k == n_k - 1))
                    nc.vector.tensor_copy(
                        v_aug[:, jt, :, 0:64],
                        psv[:].rearrange("q (h d) -> q h d", d=D))

                if phases < 2:
                    for jt in range(n_jt):
                        emit_v(jt)
                    nc.sync.dma_start(
                        out=outp_d.ap()[0:128, 0:65], in_=v_aug[:, 0, 0, :])
                    return

                next_v = [0]

                def need_v(jt):
                    while next_v[0] <= jt:
                        emit_v(next_v[0])
                        next_v[0] += 1

                # ---- phase 2: attention per (pair, head) ----
                ysb = ysbp.tile([128, n_pairs, T], BF16)
                for p in range(n_pairs):
                    for h in range(2):
                        hh = p * 2 + h
                        hs = slice(h * 64, (h + 1) * 64)
                        y_ps = ps_y.tile([65, T], F32, tag="y")
                        chunks = [(jt, c)
                                  for jt in range(n_jt)
                                  for c in range((128 * jt) // CH, T // CH)]
                        es_tiles = {}
                        pend = None

                        def emit_pv(jt, c):
                            es = es_tiles[jt]
                            lo = 128 * jt
                            s0 = jt // jt_per_strip
                            cw_lo = max(lo, c * CH)
                            full_start = (s0 + 1 if cw_lo == lo
                                          else (c * CH) // n_strip)
                            for s in range(full_start, (c + 1) * CH // n_strip):
                                nc.tensor.matmul(
                                    y_ps[:, s * n_strip:(s + 1) * n_strip],
                                    v_aug[:, jt, hh, :],
                                    es[:, s * n_strip:(s + 1) * n_strip],
                                    start=(jt == 0),
                                    stop=(jt == (s + 1) * jt_per_strip - 1),
                                    skip_group_check=True)
                            if cw_lo == lo:
                                off = 128 * jt - n_strip * s0
                                pv_n = min(n_strip - off, T - lo)
                                nc.tensor.matmul(
                                    y_ps[:, lo:lo + pv_n],
                                    v_aug[:, jt, hh, :], es[:, lo:lo + pv_n],
                                    start=(jt == 0),
                                    stop=(jt == (s0 + 1) * jt_per_strip - 1),
                                    skip_group_check=True)

                        for (jt, c) in chunks:
                            lo = 128 * jt
                            cw_lo = max(lo, c * CH)
                            if jt not in es_tiles:
                                es_tiles[jt] = esp.tile(
                                    [128, T], BF16, tag="es", name=f"es{jt % 3}")
                                if len(es_tiles) > 3:
                                    del es_tiles[min(k2 for k2 in es_tiles
                                                     if k2 != jt)]
                            es = es_tiles[jt]
                            scores = ps_a.tile([128, CH], F32, tag="a")
                            s_first = cw_lo // n_strip
                            for s in range(s_first, (c + 1) * CH // n_strip):
                                a = max(cw_lo, s * n_strip)
                                n = (s + 1) * n_strip - a
                                nc.tensor.matmul(
                                    scores[:, a - c * CH:a - c * CH + n],
                                    kt[p][hs, lo:lo + 128],
                                    qt[p][hs, a:a + n],
                                    start=True, stop=True)
                            nc.scalar.activation(
                                out=es[:, cw_lo:(c + 1) * CH],
                                in_=scores[:, cw_lo - c * CH:CH],
                                func=mybir.ActivationFunctionType.Exp)
                            if cw_lo == lo:
                                nc.vector.tensor_mul(
                                    es[:, lo:lo + 128], es[:, lo:lo + 128],
                                    mask_sb[:])
                            if pend is not None:
                                need_v(pend[0])
                                emit_pv(*pend)
                            pend = (jt, c)
                        need_v(pend[0])
                        emit_pv(*pend)

                        # fast drain to SBUF, then normalize off-PSUM
                        ydr = normp.tile([65, T], F32, tag="ydr")
                        nc.vector.tensor_copy(ydr[:], y_ps[:])
                        recip = normp.tile([1, T], F32, tag="recip")
                        nc.vector.reciprocal(recip[:], ydr[64:65, :])
                        bcast = normp.tile([64, T], F32, tag="bcast")
                        nc.gpsimd.partition_broadcast(bcast[:], recip[:])
                        nc.vector.tensor_mul(
                            ysb[h * 64:(h + 1) * 64, p, :], ydr[0:64, :],
                            bcast[:])

                if phases < 3:
                    for p in range(n_pairs):
                        nc.sync.dma_start(
                            out=outp_d.ap()[p * 128:(p + 1) * 128, :],
                            in_=ysb[:, p, :])
                    return

                # ---- phase 3: output projection; one 512KB DMA per m-tile
                # on the ACT HWDGE ring ----
                for m in range(C // 128):
                    osb = osbp.tile([128, T], BF16, tag="osb")
                    for s2 in range(n_ts // 2):
                        pso = ps_a.tile([128, 2 * n_strip], F32, tag="a")
                        for half in range(2):
                            hsl = slice(half * n_strip, (half + 1) * n_strip)
                            ssl = slice((s2 * 2 + half) * n_strip,
                                        (s2 * 2 + half + 1) * n_strip)
                            for kp in range(CIN // 128):
                                nc.tensor.matmul(
                                    pso[:, hsl],
                                    wp_sb[:, kp, m * 128:(m + 1) * 128],
                                    ysb[:, kp, ssl],
                                    start=(kp == 0), stop=(kp == CIN // 128 - 1))
                        dsl = slice(s2 * 2 * n_strip, (s2 + 1) * 2 * n_strip)
                        if s2 % 2 == 0:
                            nc.vector.tensor_copy(osb[:, dsl], pso[:])
                        else:
                            nc.scalar.copy(osb[:, dsl], pso[:])
                    nc.scalar.dma_start(
                        out=outp_d.ap()[m * 128:(m + 1) * 128, :], in_=osb[:])

            if reps == 1:
                body()
            else:
                with tc.For_i(0, reps, 1) as i:
                    body(i)
    nc.compile()
    return nc


def host_inputs(x, W_attn, W_proj, n_groups=N_GROUPS):
    """Per-core input maps. Core order: g * B + b."""
    B, T, C = x.shape
    hp = N_HEAD // n_groups
    n_pairs = hp // 2
    scale = np.float32(1.0 / np.sqrt(D))
    mask = (np.arange(128)[None, :] >= np.arange(128)[:, None]).astype(BF16_NP)
    in_maps = []
    for g in range(n_groups):
        qk_cols, v_cols = [], []
        for p in range(n_pairs):
            h0 = g * hp + 2 * p
            h1 = h0 + 1
            qk_cols.append(W_attn[:, h0 * D:(h0 + 1) * D] * scale)
            qk_cols.append(W_attn[:, h1 * D:(h1 + 1) * D] * scale)
            qk_cols.append(W_attn[:, C + h0 * D:C + (h0 + 1) * D])
            qk_cols.append(W_attn[:, C + h1 * D:C + (h1 + 1) * D])
            v_cols.append(W_attn[:, 2 * C + h0 * D:2 * C + (h0 + 1) * D])
            v_cols.append(W_attn[:, 2 * C + h1 * D:2 * C + (h1 + 1) * D])
        wqk = np.ascontiguousarray(
            np.concatenate(qk_cols, axis=1)).astype(BF16_NP)
        wv = np.ascontiguousarray(
            np.concatenate(v_cols, axis=1)).astype(BF16_NP)
        wp = np.ascontiguousarray(
            W_proj[g * hp * D:(g + 1) * hp * D]).astype(BF16_NP)
        for b in range(B):
            xt = np.ascontiguousarray(x[b].T).astype(BF16_NP)
            in_maps.append({"xt": xt, "wqk": wqk, "wv": wv, "wp": wp, "mask": mask})
    return in_maps


def host_gather(results, B, T, C, n_groups=N_GROUPS):
    out = np.zeros((B, T, C), dtype=np.float32)
    for g in range(n_groups):
        for b in range(B):
            out[b] += results[g * B + b]["outp"].T.astype(np.float32)
    return out


_NC_CACHE = {}


def kernel(x, W_attn, W_proj):
    x = np.asarray(x, dtype=np.float32)
    W_attn = np.asarray(W_attn, dtype=np.float32)
    W_proj = np.asarray(W_proj, dtype=np.float32)
    B, T, C = x.shape
    if "nc" not in _NC_CACHE:
        _NC_CACHE["nc"] = build_kernel(T=T, C=C)
    nc = _NC_CACHE["nc"]
    in_maps = host_inputs(x, W_attn, W_proj)
    res = bass_utils.run_bass_kernel_spmd(nc, in_maps, core_ids=list(range(8)))
    return host_gather(res.results, B, T, C)
